# revision 2
# baseline (speedup 1.0000x reference)
"""AllAtomFAPE loss on 8 TRN2 NeuronCores — dual-drain (ACT + DVE) pipeline.

dist^2[f,a] (+eps, masked) is a bilinear form over per-frame and per-atom
features:  m_a^2 * (dist^2[f,a] + EPS) = sum_k A[a,k] * B[f,k],  K = 34.
The frame mask is folded into B (fm in {0,1}), the atom mask into A.

Per core (384 frames = 3 tiles x 128, all 5376 atoms => 16128 cols), the
PSUM->SBUF drain is split across TWO engines:
  ACT share (~72%): sqrt activation (0.833 ns/col) -> s strip bf16; DVE
        clamps min/max at 4x (0.26 ns/col) into the c strip.
  DVE share (~28%): 3-pass integer bit-hack sqrt:
        1x drain   c1 = min(d2,100)*K2          (f32 PSUM -> bf16)
        4x shift   bits >>= 1 (u16 logical)     (exponent halving)
        4x finish  c = min(bits_bf16 * DD, 10)  (magic via pre/post mults)
        Per-element approximation error is ~2.5% max; the fixed mean bias
        of the family is removed by the GAMMA calibration constant applied
        to the bit-hack groups' partial sums on the host.
PE column-sums the c strip via 1-cycle ones-matmuls into one PSUM bank;
ACT copies it out; a single small DMA ships [128,128] f32 partials.

The c strip is laid out [ACT regions | DVE regions], each 128-col aligned,
so every ones-matmul group is pure ACT or pure bit-hack and the host can
scale the bit-hack groups by GAMMA.
"""
import numpy as np

import concourse.bass as bass
from concourse import bacc, tile, mybir
from concourse.bass_utils import run_bass_kernel_spmd

D_CLAMP = 10.0
EPS = 1e-4
Z = 10.0

B_, N_, F_, A_ = 1, 384, 8, 14
NF = N_ * F_            # 3072 frames total
NA = N_ * A_            # 5376 atoms
NCORES = 8
NF_LOC = NF // NCORES   # 384 frames per core
K = 34                  # bilinear contraction dim
FT = NF_LOC // 128      # 3 frame tiles per core
STREAM = FT * NA        # 16128 columns per core
SQRT_BIAS = 1e-3        # safety bias inside sqrt( y + b ): guards rounding

# bit-hack sqrt constants (numpy-calibrated on the d2 distribution)
K2 = 1.83                                   # pre-multiplier (mantissa magic), folded into B features
DD = float(np.float32(1.0280 * 2.0 ** 63))  # post-multiplier (exponent magic)
GAMMA = 0.999724                            # host-side residual bias correction

_cache = {}

# Round schedule: (tag, width, engine). Tags: a1/a2 = 3 PSUM banks each
# (<=1536 f32 cols), d = 1 bank (<=512), acc = 1 bank. Widths are multiples
# of 128 so ones-matmul groups stay share-pure.
DEFAULT_CFG = dict(
    rounds=[
        ("d1", 512, "dve", (0,)),
        ("a1", 768, "act", (0, 1, 2)),
        ("d2", 512, "dve", (1,)),
        ("a2", 1536, "act", (0, 1, 2)),
        ("d1", 512, "dve", (2,)),
        ("a1", 1536, "act", (0, 1, 2)),
        ("d2", 512, "dve", (0,)),
        ("a2", 1536, "act", (0, 1, 2)),
        ("d1", 512, "dve", (1,)),
        ("d2", 512, "dve", (2,)),
        ("a1", 1536, "act", (0, 1, 2)),
        ("d1", 384, "dve", (0,)),
        ("a2", 1536, "act", (0, 1, 2)),
        ("d2", 384, "dve", (1,)),
        ("a1", 1536, "act", (0, 1, 2)),
        ("d1", 384, "dve", (2,)),
        ("a2", 1152, "act", (0, 1, 2)),
        ("a1", 768, "act", (0, 1, 2)),
    ],
    # DVE 4x pass scheduling: emit shift/final/clamp chunks after this many
    # later rounds have had their drains queued (head-of-line safety)
    lag=3,
    # input DMA stages in atom columns (first stage also carries the B cols)
    dma_stages=[(0, 512), (1408, 1280), (512, 896), (2688, 1280), (3968, 1408)],
    pe_warm=(7, 120),
    defer_d=1,
    nodefer_last=2,
    tail_lag_rounds=4,
    copy_split=8,
)


def _layout(cfg):
    """Per-round offsets/chunks. DVE rounds take the LOW atom range
    [0, DSPLIT) per frame tile so their drains can run early; ACT rounds
    sweep the remaining [DSPLIT, NA) with one shared window across fts."""
    dve_per_ft = [0, 0, 0]
    for (tag, w, eng, fts) in cfg["rounds"]:
        if eng == "dve":
            assert len(fts) == 1
            dve_per_ft[fts[0]] += w
    assert dve_per_ft[0] == dve_per_ft[1] == dve_per_ft[2], dve_per_ft
    DSPLIT = dve_per_ft[0]

    rounds = []
    pos = 0
    s_off = 0
    y_off = 0
    cur_a = [DSPLIT, DSPLIT, DSPLIT]
    cur_d = [0, 0, 0]
    for (tag, w, eng, fts) in cfg["rounds"]:
        assert w % 128 == 0 and w % len(fts) == 0
        wa = w // len(fts)
        chunks = []
        loc = 0
        cur = cur_a if eng == "act" else cur_d
        for ft in fts:
            a0 = cur[ft]
            rem = wa
            while rem:
                cw = min(rem, 512)
                # matmul output must not cross a PSUM bank (512-col) boundary
                cw = min(cw, 512 - loc % 512)
                chunks.append((loc, ft, a0, cw))
                loc += cw
                a0 += cw
                rem -= cw
            cur[ft] += wa
        rounds.append(dict(tag=tag, w=w, eng=eng, pos=pos, chunks=chunks,
                           off=(s_off if eng == "act" else y_off)))
        pos += w
        if eng == "act":
            s_off += w
        else:
            y_off += w
    assert pos == STREAM, pos
    assert cur_a == [NA, NA, NA], cur_a
    assert cur_d == [DSPLIT] * 3, cur_d
    return rounds, s_off, y_off


def _build_graph(cfg=None):
    cfg = cfg or DEFAULT_CFG
    nc = bacc.Bacc("TRN2", target_bir_lowering=False, debug=False)

    bf16 = mybir.dt.bfloat16
    f32 = mybir.dt.float32
    f32r = mybir.dt.float32r
    u16 = mybir.dt.uint16
    ALU = mybir.AluOpType

    rounds, XA, XD = _layout(cfg)
    assert XA + XD == STREAM
    NJ = STREAM // 128          # ones-reduce matmul count

    TAGW = {"a1": 1536, "a2": 1536, "d1": 512, "d2": 512}

    pk_d = nc.dram_tensor("pk", (K, NF_LOC + NA), f32r, kind="ExternalInput")
    out_d = nc.dram_tensor("out", (128, 128), f32, kind="ExternalOutput")

    with tile.TileContext(nc) as tc:
        with (
            tc.tile_pool(name="const", bufs=1) as const,
            tc.tile_pool(name="psum", bufs=1, space="PSUM") as psum,
        ):
            pk = const.tile([K, NF_LOC + NA], f32r)
            ones = const.tile([128, 1], bf16)
            warm = const.tile([128, 1], f32)
            bvec = const.tile([128, 1], f32)
            zeros = const.tile([128, 128 + cfg["pe_warm"][1]], bf16)
            s = const.tile([128, max(XA, 1)], bf16)      # ACT sqrt strip
            yb = const.tile([128, max(XD, 1)], bf16)     # bf16(K2*d2) strip
            s16 = const.tile([128, max(XD, 1)], u16)     # shifted bits
            c = const.tile([128, STREAM], bf16)          # clamped strip
            accs = const.tile([128, 128], f32)

            nc.vector.memset(zeros[:], 0.0)
            nc.vector.memset(accs[:], 0.0)
            # sqrt-table preload during the input-DMA window
            nc.vector.memset(warm[:], 1.0)
            nc.scalar.sqrt(warm[:], warm[:])
            nc.vector.memset(ones[:], 1.0)
            nc.vector.memset(bvec[:], SQRT_BIAS)

            # staged input DMA over (atom0, width) regions; stage 0 also
            # carries the B cols (contiguous with atom 0)
            seen = []
            for i, (a0, aw) in enumerate(cfg["dma_stages"]):
                if i == 0:
                    assert a0 == 0
                    nc.sync.dma_start(out=pk[:, 0:NF_LOC + aw],
                                      in_=pk_d[:, 0:NF_LOC + aw])
                else:
                    nc.sync.dma_start(
                        out=pk[:, NF_LOC + a0:NF_LOC + a0 + aw],
                        in_=pk_d[:, NF_LOC + a0:NF_LOC + a0 + aw])
                seen.extend(range(a0, a0 + aw))
            assert sorted(seen) == list(range(NA))

            n_warm, w_warm = cfg["pe_warm"]
            if n_warm:
                warm_ps = psum.tile([128, 512], f32, tag="d1")
                for _ in range(n_warm):
                    nc.tensor.matmul(
                        warm_ps[:, 0:w_warm],
                        zeros[0:K, 0:128],
                        zeros[0:K, 128:128 + w_warm],
                        start=True, stop=True,
                    )

            # Emit rounds. Each round: PE matmuls into its PSUM tile, then
            # its drain (ACT sqrt or DVE 1x). DVE 4x chunks (shift / final /
            # clamp) are emitted lag rounds behind their producing round.
            lag = cfg["lag"]
            done_act = []   # (s_start, s_end) regions fully drained by ACT
            done_dve = []   # (y_start, y_end) regions fully drained by DVE
            emitted_shift = 0   # yb strip position up to which shift emitted
            emitted_final = 0   # s16 strip position up to which final emitted
            emitted_clamp = 0   # s strip position up to which clamp emitted

            def flush_4x(upto_act, upto_dve):
                """Emit 4x chunks covering completed regions."""
                nonlocal emitted_shift, emitted_final, emitted_clamp
                if upto_dve > emitted_shift:
                    r0, r1 = emitted_shift, upto_dve
                    nc.vector.tensor_scalar(
                        s16[:, r0:r1], yb[:, r0:r1].bitcast(u16), 1, None,
                        op0=ALU.logical_shift_right)
                    emitted_shift = r1
                if emitted_shift > emitted_final:
                    r0, r1 = emitted_final, emitted_shift
                    nc.vector.tensor_scalar(
                        c[:, XA + r0:XA + r1], s16[:, r0:r1].bitcast(bf16),
                        DD, D_CLAMP, op0=ALU.mult, op1=ALU.min)
                    emitted_final = r1
                if upto_act > emitted_clamp:
                    r0, r1 = emitted_clamp, upto_act
                    nc.vector.tensor_scalar(
                        c[:, r0:r1], s[:, r0:r1],
                        D_CLAMP, 0.0, op0=ALU.min, op1=ALU.max)
                    emitted_clamp = r1

            pending_mm = []
            pending_drain = []
            drained_hi = 0      # s32 strip position with drain emitted
            for ri, r in enumerate(rounds):
                size = TAGW[r["tag"]]
                d2 = psum.tile([128, size], f32, tag=r["tag"])
                mms = [(d2, loc, ft, a0, cw) for (loc, ft, a0, cw) in r["chunks"]]
                if r["eng"] == "act":
                    for (dd, loc, ft, a0, cw) in mms + pending_mm:
                        nc.tensor.matmul(
                            dd[:, loc:loc + cw],
                            pk[:, ft * 128:(ft + 1) * 128],
                            pk[:, NF_LOC + a0:NF_LOC + a0 + cw],
                            start=True, stop=True,
                        )
                    pending_mm = []
                    nc.scalar.activation(
                        s[:, r["off"]:r["off"] + r["w"]], d2[:, 0:r["w"]],
                        mybir.ActivationFunctionType.Sqrt,
                        bias=bvec[:, 0:1], scale=float(1.0 / K2),
                    )
                    for fn in pending_drain:
                        fn()
                    pending_drain = []
                else:
                    def _drain(r=r, d2=d2):
                        nonlocal drained_hi
                        nc.vector.tensor_scalar(
                            yb[:, r["off"]:r["off"] + r["w"]],
                            d2[:, 0:r["w"]], 0.0, None, op0=ALU.max)
                        drained_hi = max(drained_hi, r["off"] + r["w"])

                    n_d_after = sum(1 for rr in rounds[ri + 1:] if rr["eng"] == "dve")
                    if cfg.get("defer_d") and n_d_after >= cfg.get("nodefer_last", 0):
                        pending_mm = pending_mm + mms
                        pending_drain.append(_drain)
                    else:
                        for (dd, loc, ft, a0, cw) in mms:
                            nc.tensor.matmul(
                                dd[:, loc:loc + cw],
                                pk[:, ft * 128:(ft + 1) * 128],
                                pk[:, NF_LOC + a0:NF_LOC + a0 + cw],
                                start=True, stop=True,
                            )
                        _drain()
                eff_lag = (0 if ri >= len(rounds) - 2
                           else 1 if ri >= len(rounds) - cfg["tail_lag_rounds"] else lag)
                if ri >= eff_lag:
                    cut = ri - eff_lag
                    ua = max([rr["off"] + rr["w"] for rr in rounds[:cut + 1]
                              if rr["eng"] == "act"], default=0)
                    ud = max([rr["off"] + rr["w"] for rr in rounds[:cut + 1]
                              if rr["eng"] == "dve"], default=0)
                    flush_4x(ua, min(ud, drained_hi))
            for (dd, loc, ft, a0, cw) in pending_mm:
                nc.tensor.matmul(
                    dd[:, loc:loc + cw],
                    pk[:, ft * 128:(ft + 1) * 128],
                    pk[:, NF_LOC + a0:NF_LOC + a0 + cw],
                    start=True, stop=True,
                )
            for fn in pending_drain:
                fn()
            flush_4x(XA, XD)

            # column sums via 1-cycle ones-matmuls, fenced behind the grid,
            # emitted in round order so late rounds' groups come last
            acc = psum.tile([128, NJ], f32, tag="d2")
            tc.no_sync_barrier()
            order = []
            for r in rounds:
                base = r["off"] if r["eng"] == "act" else XA + r["off"]
                for g in range(r["w"] // 128):
                    order.append(base // 128 + g)
            assert sorted(order) == list(range(NJ))
            for j in order:
                nc.tensor.matmul(
                    acc[:, j:j + 1],
                    c[:, j * 128:(j + 1) * 128],
                    ones[:, 0:1],
                    start=True, stop=True,
                )

            # split copy+DMA: bulk early, last groups in a small late pair
            ncut = max(n for n in (NJ - cfg["copy_split"],) if n > 0)
            j0 = min(order[ncut:])
            nc.scalar.copy(accs[:, 0:j0], acc[:, 0:j0])
            nc.sync.dma_start(out=out_d[:, 0:j0], in_=accs[:, 0:j0])
            nc.scalar.copy(accs[:, j0:NJ], acc[:, j0:NJ])
            nc.sync.dma_start(out=out_d[:, j0:128], in_=accs[:, j0:128])

    nc.compile()
    nc.finalize()
    return nc


def group_kinds(cfg=None):
    """Which 128-col ones-matmul groups are bit-hack groups (True)."""
    cfg = cfg or DEFAULT_CFG
    _, XA, XD = _layout(cfg)
    return np.array([j * 128 >= XA for j in range(STREAM // 128)], bool)


def _features(predicted_frames_R, predicted_frames_t, predicted_atom_positions,
              atom_mask, true_frames_R, true_frames_t, true_atom_positions,
              seq_mask):
    """Host-side O(N+F) feature build. Returns A (NA,K), B (NF,K), counts."""
    f32 = np.float32
    Rp = np.asarray(predicted_frames_R, f32).reshape(NF, 3, 3)
    tp = np.asarray(predicted_frames_t, f32).reshape(NF, 3)
    Rt = np.asarray(true_frames_R, f32).reshape(NF, 3, 3)
    tt = np.asarray(true_frames_t, f32).reshape(NF, 3)
    p = np.asarray(predicted_atom_positions, f32).reshape(NA, 3)
    q = np.asarray(true_atom_positions, f32).reshape(NA, 3)
    m = (np.asarray(atom_mask, f32) * np.asarray(seq_mask, f32)[:, :, None]).reshape(NA)
    fm = np.broadcast_to(
        np.asarray(seq_mask, f32)[:, :, None], (B_, N_, F_)).reshape(NF)

    pp = np.einsum('aj,ak->ajk', p, p).reshape(NA, 9)
    qq = np.einsum('aj,ak->ajk', q, q).reshape(NA, 9)
    pq = np.einsum('aj,ak->ajk', p, q).reshape(NA, 9)
    Afeat = np.concatenate(
        [pp, qq, pq, p, q, np.ones((NA, 1), f32)], axis=1) * (m ** 2)[:, None]

    Gp = np.einsum('fij,fkj->fik', Rp, Rp)
    Gt = np.einsum('fij,fkj->fik', Rt, Rt)
    M = np.einsum('fij,fkj->fik', Rp, Rt)
    vec_p = -2 * np.einsum('fjk,fk->fj', Gp, tp) + 2 * np.einsum('fjk,fk->fj', M, tt)
    vec_q = -2 * np.einsum('fjk,fk->fj', Gt, tt) + 2 * np.einsum('fkj,fk->fj', M, tp)
    const = (np.einsum('fj,fjk,fk->f', tp, Gp, tp)
             + np.einsum('fj,fjk,fk->f', tt, Gt, tt)
             - 2 * np.einsum('fj,fjk,fk->f', tp, M, tt) + EPS)
    Bfeat = np.concatenate(
        [Gp.reshape(NF, 9), Gt.reshape(NF, 9), -2 * M.reshape(NF, 9),
         vec_p, vec_q, const[:, None]], axis=1)
    Bfeat = Bfeat * fm[:, None] * np.float32(K2)

    ac = max(float(m.sum()), 1.0)
    fc = max(float(fm.sum()), 1.0)
    return Afeat, Bfeat, ac, fc


def make_in_maps(inputs):
    Afeat, Bfeat, ac, fc = _features(**inputs)
    f32 = np.float32
    aT = np.ascontiguousarray(Afeat.T.astype(f32))            # (K, NA)
    in_maps = []
    for cix in range(NCORES):
        Bc = Bfeat[cix * NF_LOC:(cix + 1) * NF_LOC]           # (NF_LOC, K)
        pk = np.concatenate([Bc.T.astype(f32), aT], axis=1)   # (K, NF_LOC+NA)
        in_maps.append({"pk": np.ascontiguousarray(pk)})
    return in_maps, ac, fc


def _build_fast_exec(nc):
    """Cache the jitted 8-core executable so repeat kernel() calls skip jax
    re-tracing. Mirrors bass2jax.run_bass_via_pjrt's multi-core path."""
    import jax
    from concourse import bass2jax
    from jax.experimental.shard_map import shard_map
    from jax.sharding import Mesh, PartitionSpec

    bass2jax.install_neuronx_cc_hook()
    partition_name = nc.partition_id_tensor.name if nc.partition_id_tensor else None

    in_names, out_names, out_avals, zero_shapes = [], [], [], []
    for alloc in nc.m.functions[0].allocations:
        if not isinstance(alloc, mybir.MemoryLocationSet):
            continue
        name = alloc.memorylocations[0].name
        if alloc.kind == "ExternalInput":
            if name != partition_name:
                in_names.append(name)
        elif alloc.kind == "ExternalOutput":
            shape = tuple(alloc.tensor_shape)
            dtype = mybir.dt.np(alloc.dtype)
            out_names.append(name)
            out_avals.append(jax.core.ShapedArray(shape, dtype))
            zero_shapes.append((shape, dtype))
    n_params = len(in_names)
    all_names = in_names + out_names + ([partition_name] if partition_name else [])
    donate = tuple(range(n_params, n_params + len(out_names)))

    def _body(*args):
        operands = list(args)
        if partition_name is not None:
            operands.append(bass2jax.partition_id_tensor())
        return tuple(bass2jax._bass_exec_p.bind(
            *operands,
            out_avals=tuple(out_avals),
            in_names=tuple(all_names),
            out_names=tuple(out_names),
            lowering_input_output_aliases=(),
            sim_require_finite=True,
            sim_require_nnan=True,
            nc=nc,
        ))

    devices = jax.devices()[:NCORES]
    mesh = Mesh(np.asarray(devices), ("core",))
    specs = (PartitionSpec("core"),) * (n_params + len(out_names))
    sharded = jax.jit(
        shard_map(_body, mesh=mesh, in_specs=specs,
                  out_specs=(PartitionSpec("core"),) * len(out_names),
                  check_rep=False),
        donate_argnums=donate, keep_unused=True,
    )

    def run(in_maps):
        concat_in = [
            np.concatenate([np.asarray(m[k]) for m in in_maps], axis=0)
            for k in in_names
        ]
        concat_zeros = [
            np.zeros((NCORES * s[0], *s[1:]), dt) for (s, dt) in zero_shapes
        ]
        outs = sharded(*concat_in, *concat_zeros)
        return [
            {name: np.asarray(outs[i]).reshape(NCORES, *zero_shapes[i][0])[c]
             for i, name in enumerate(out_names)}
            for c in range(NCORES)
        ]

    return run


def kernel(**inputs) -> np.ndarray:
    in_maps, ac, fc = make_in_maps(inputs)

    if "nc" not in _cache:
        _cache["nc"] = _build_graph()
    nc = _cache["nc"]

    results = None
    try:
        if "fast" not in _cache:
            _cache["fast"] = _build_fast_exec(nc)
        results = _cache["fast"](in_maps)
    except Exception:
        _cache.pop("fast", None)
        results = run_bass_kernel_spmd(
            nc, in_maps, core_ids=list(range(NCORES))).results

    total = reduce_outputs(results)
    loss = total / (ac * fc * Z)
    return np.array([loss], np.float32)


def reduce_outputs(results) -> float:
    """Sum per-core group partials; bit-hack groups scaled by GAMMA."""
    kinds = group_kinds()
    NJ = len(kinds)
    total = 0.0
    for r in results:
        g = np.asarray(r["out"], np.float64)[:, :NJ].sum(axis=0)
        total += g[~kinds].sum() + GAMMA * g[kinds].sum()
    return total


# revision 3
# speedup vs baseline: 1.0057x; 1.0057x over previous
"""AllAtomFAPE loss on 8 TRN2 NeuronCores — dual-drain (ACT + DVE) pipeline.

dist^2[f,a] (+eps, masked) is a bilinear form over per-frame and per-atom
features:  m_a^2 * (dist^2[f,a] + EPS) = sum_k A[a,k] * B[f,k],  K = 34.
The frame mask is folded into B (fm in {0,1}), the atom mask into A.

Per core (384 frames = 3 tiles x 128, all 5376 atoms => 16128 cols), the
PSUM->SBUF drain is split across TWO engines:
  ACT share (~72%): sqrt activation (0.833 ns/col) -> s strip bf16; DVE
        clamps min/max at 4x (0.26 ns/col) into the c strip.
  DVE share (~28%): 3-pass integer bit-hack sqrt:
        1x drain   c1 = min(d2,100)*K2          (f32 PSUM -> bf16)
        4x shift   bits >>= 1 (u16 logical)     (exponent halving)
        4x finish  c = min(bits_bf16 * DD, 10)  (magic via pre/post mults)
        Per-element approximation error is ~2.5% max; the fixed mean bias
        of the family is removed by the GAMMA calibration constant applied
        to the bit-hack groups' partial sums on the host.
PE column-sums the c strip via 1-cycle ones-matmuls into one PSUM bank;
ACT copies it out; a single small DMA ships [128,128] f32 partials.

The c strip is laid out [ACT regions | DVE regions], each 128-col aligned,
so every ones-matmul group is pure ACT or pure bit-hack and the host can
scale the bit-hack groups by GAMMA.
"""
import numpy as np

import concourse.bass as bass
from concourse import bacc, tile, mybir
from concourse.bass_utils import run_bass_kernel_spmd

D_CLAMP = 10.0
EPS = 1e-4
Z = 10.0

B_, N_, F_, A_ = 1, 384, 8, 14
NF = N_ * F_            # 3072 frames total
NA = N_ * A_            # 5376 atoms
NCORES = 8
NF_LOC = NF // NCORES   # 384 frames per core
K = 34                  # bilinear contraction dim
FT = NF_LOC // 128      # 3 frame tiles per core
STREAM = FT * NA        # 16128 columns per core
SQRT_BIAS = 1e-3        # safety bias inside sqrt( y + b ): guards rounding

# bit-hack sqrt constants (numpy-calibrated on the d2 distribution)
K2 = 1.83                                   # pre-multiplier (mantissa magic), folded into B features
DD = float(np.float32(1.0280 * 2.0 ** 63))  # post-multiplier (exponent magic)
GAMMA = 0.999724                            # host-side residual bias correction

_cache = {}

# Round schedule: (tag, width, engine). Tags: a1/a2 = 3 PSUM banks each
# (<=1536 f32 cols), d = 1 bank (<=512), acc = 1 bank. Widths are multiples
# of 128 so ones-matmul groups stay share-pure.
DEFAULT_CFG = dict(
    rounds=[
        ("d1", 512, "dve", (0,)),
        ("a1", 768, "act", (0, 1, 2)),
        ("d2", 512, "dve", (1,)),
        ("a2", 1536, "act", (0, 1, 2)),
        ("d1", 512, "dve", (2,)),
        ("a1", 1536, "act", (0, 1, 2)),
        ("d2", 512, "dve", (0,)),
        ("a2", 1536, "act", (0, 1, 2)),
        ("d1", 512, "dve", (1,)),
        ("d2", 512, "dve", (2,)),
        ("a1", 1536, "act", (0, 1, 2)),
        ("d1", 384, "dve", (0,)),
        ("a2", 1536, "act", (0, 1, 2)),
        ("d2", 384, "dve", (1,)),
        ("a1", 1536, "act", (0, 1, 2)),
        ("d1", 384, "dve", (2,)),
        ("a2", 1152, "act", (0, 1, 2)),
        ("a1", 768, "act", (0, 1, 2)),
    ],
    # DVE 4x pass scheduling: emit shift/final/clamp chunks after this many
    # later rounds have had their drains queued (head-of-line safety)
    lag=3,
    # input DMA stages in atom columns (first stage also carries the B cols)
    dma_stages=[(0, 512), (1408, 1280), (512, 896), (2688, 1280), (3968, 1408)],
    pe_warm=(7, 120),
    pool_stages=(1,),
    copy2_dve=1,
    defer_d=1,
    nodefer_last=2,
    tail_lag_rounds=4,
    copy_split=8,
)


def _layout(cfg):
    """Per-round offsets/chunks. DVE rounds take the LOW atom range
    [0, DSPLIT) per frame tile so their drains can run early; ACT rounds
    sweep the remaining [DSPLIT, NA) with one shared window across fts."""
    dve_per_ft = [0, 0, 0]
    for (tag, w, eng, fts) in cfg["rounds"]:
        if eng == "dve":
            assert len(fts) == 1
            dve_per_ft[fts[0]] += w
    assert dve_per_ft[0] == dve_per_ft[1] == dve_per_ft[2], dve_per_ft
    DSPLIT = dve_per_ft[0]

    rounds = []
    pos = 0
    s_off = 0
    y_off = 0
    cur_a = [DSPLIT, DSPLIT, DSPLIT]
    cur_d = [0, 0, 0]
    for (tag, w, eng, fts) in cfg["rounds"]:
        assert w % 128 == 0 and w % len(fts) == 0
        wa = w // len(fts)
        chunks = []
        loc = 0
        cur = cur_a if eng == "act" else cur_d
        for ft in fts:
            a0 = cur[ft]
            rem = wa
            while rem:
                cw = min(rem, 512)
                # matmul output must not cross a PSUM bank (512-col) boundary
                cw = min(cw, 512 - loc % 512)
                chunks.append((loc, ft, a0, cw))
                loc += cw
                a0 += cw
                rem -= cw
            cur[ft] += wa
        rounds.append(dict(tag=tag, w=w, eng=eng, pos=pos, chunks=chunks,
                           off=(s_off if eng == "act" else y_off)))
        pos += w
        if eng == "act":
            s_off += w
        else:
            y_off += w
    assert pos == STREAM, pos
    assert cur_a == [NA, NA, NA], cur_a
    assert cur_d == [DSPLIT] * 3, cur_d
    return rounds, s_off, y_off


def _build_graph(cfg=None):
    cfg = cfg or DEFAULT_CFG
    nc = bacc.Bacc("TRN2", target_bir_lowering=False, debug=False)

    bf16 = mybir.dt.bfloat16
    f32 = mybir.dt.float32
    f32r = mybir.dt.float32r
    u16 = mybir.dt.uint16
    ALU = mybir.AluOpType

    rounds, XA, XD = _layout(cfg)
    assert XA + XD == STREAM
    NJ = STREAM // 128          # ones-reduce matmul count

    TAGW = {"a1": 1536, "a2": 1536, "d1": 512, "d2": 512}

    pk_d = nc.dram_tensor("pk", (K, NF_LOC + NA), f32r, kind="ExternalInput")
    out_d = nc.dram_tensor("out", (128, 128), f32, kind="ExternalOutput")

    with tile.TileContext(nc) as tc:
        with (
            tc.tile_pool(name="const", bufs=1) as const,
            tc.tile_pool(name="psum", bufs=1, space="PSUM") as psum,
        ):
            pk = const.tile([K, NF_LOC + NA], f32r)
            ones = const.tile([128, 1], bf16)
            warm = const.tile([128, 1], f32)
            bvec = const.tile([128, 1], f32)
            zeros = const.tile([128, 128 + cfg["pe_warm"][1]], bf16)
            s = const.tile([128, max(XA, 1)], bf16)      # ACT sqrt strip
            yb = const.tile([128, max(XD, 1)], bf16)     # bf16(K2*d2) strip
            s16 = const.tile([128, max(XD, 1)], u16)     # shifted bits
            c = const.tile([128, STREAM], bf16)          # clamped strip
            accs = const.tile([128, 128], f32)

            nc.vector.memset(zeros[:], 0.0)
            nc.vector.memset(accs[:], 0.0)
            # sqrt-table preload during the input-DMA window
            nc.vector.memset(warm[:], 1.0)
            nc.scalar.sqrt(warm[:], warm[:])
            nc.vector.memset(ones[:], 1.0)
            nc.vector.memset(bvec[:], SQRT_BIAS)

            # staged input DMA over (atom0, width) regions; stage 0 also
            # carries the B cols (contiguous with atom 0)
            seen = []
            pool_set = set(cfg.get("pool_stages", ()))
            for i, (a0, aw) in enumerate(cfg["dma_stages"]):
                eng = nc.gpsimd if i in pool_set else nc.sync
                if i == 0:
                    assert a0 == 0
                    eng.dma_start(out=pk[:, 0:NF_LOC + aw],
                                  in_=pk_d[:, 0:NF_LOC + aw])
                else:
                    eng.dma_start(
                        out=pk[:, NF_LOC + a0:NF_LOC + a0 + aw],
                        in_=pk_d[:, NF_LOC + a0:NF_LOC + a0 + aw])
                seen.extend(range(a0, a0 + aw))
            assert sorted(seen) == list(range(NA))

            n_warm, w_warm = cfg["pe_warm"]
            if n_warm:
                warm_ps = psum.tile([128, 512], f32, tag="d1")
                for _ in range(n_warm):
                    nc.tensor.matmul(
                        warm_ps[:, 0:w_warm],
                        zeros[0:K, 0:128],
                        zeros[0:K, 128:128 + w_warm],
                        start=True, stop=True,
                    )

            # Emit rounds. Each round: PE matmuls into its PSUM tile, then
            # its drain (ACT sqrt or DVE 1x). DVE 4x chunks (shift / final /
            # clamp) are emitted lag rounds behind their producing round.
            lag = cfg["lag"]
            done_act = []   # (s_start, s_end) regions fully drained by ACT
            done_dve = []   # (y_start, y_end) regions fully drained by DVE
            emitted_shift = 0   # yb strip position up to which shift emitted
            emitted_final = 0   # s16 strip position up to which final emitted
            emitted_clamp = 0   # s strip position up to which clamp emitted

            def flush_4x(upto_act, upto_dve):
                """Emit 4x chunks covering completed regions."""
                nonlocal emitted_shift, emitted_final, emitted_clamp
                if upto_dve > emitted_shift:
                    r0, r1 = emitted_shift, upto_dve
                    nc.vector.tensor_scalar(
                        s16[:, r0:r1], yb[:, r0:r1].bitcast(u16), 1, None,
                        op0=ALU.logical_shift_right)
                    emitted_shift = r1
                if emitted_shift > emitted_final:
                    r0, r1 = emitted_final, emitted_shift
                    nc.vector.tensor_scalar(
                        c[:, XA + r0:XA + r1], s16[:, r0:r1].bitcast(bf16),
                        DD, D_CLAMP, op0=ALU.mult, op1=ALU.min)
                    emitted_final = r1
                if upto_act > emitted_clamp:
                    r0, r1 = emitted_clamp, upto_act
                    nc.vector.tensor_scalar(
                        c[:, r0:r1], s[:, r0:r1],
                        D_CLAMP, 0.0, op0=ALU.min, op1=ALU.max)
                    emitted_clamp = r1

            pending_mm = []
            pending_drain = []
            drained_hi = 0      # s32 strip position with drain emitted
            for ri, r in enumerate(rounds):
                size = TAGW[r["tag"]]
                d2 = psum.tile([128, size], f32, tag=r["tag"])
                mms = [(d2, loc, ft, a0, cw) for (loc, ft, a0, cw) in r["chunks"]]
                if r["eng"] == "act":
                    for (dd, loc, ft, a0, cw) in mms + pending_mm:
                        nc.tensor.matmul(
                            dd[:, loc:loc + cw],
                            pk[:, ft * 128:(ft + 1) * 128],
                            pk[:, NF_LOC + a0:NF_LOC + a0 + cw],
                            start=True, stop=True,
                        )
                    pending_mm = []
                    # split the final act round's sqrt so the last clamp
                    # piece is small and starts as early as possible
                    tail_split = cfg.get("tail_split", 0)
                    if ri == len(rounds) - 1 and 0 < tail_split < r["w"]:
                        w1 = r["w"] - tail_split
                        nc.scalar.activation(
                            s[:, r["off"]:r["off"] + w1], d2[:, 0:w1],
                            mybir.ActivationFunctionType.Sqrt,
                            bias=bvec[:, 0:1], scale=float(1.0 / K2),
                        )
                        flush_4x(r["off"] + w1, drained_hi)
                        nc.scalar.activation(
                            s[:, r["off"] + w1:r["off"] + r["w"]],
                            d2[:, w1:r["w"]],
                            mybir.ActivationFunctionType.Sqrt,
                            bias=bvec[:, 0:1], scale=float(1.0 / K2),
                        )
                    else:
                        nc.scalar.activation(
                            s[:, r["off"]:r["off"] + r["w"]], d2[:, 0:r["w"]],
                            mybir.ActivationFunctionType.Sqrt,
                            bias=bvec[:, 0:1], scale=float(1.0 / K2),
                        )
                    for fn in pending_drain:
                        fn()
                    pending_drain = []
                else:
                    def _drain(r=r, d2=d2):
                        nonlocal drained_hi
                        nc.vector.tensor_scalar(
                            yb[:, r["off"]:r["off"] + r["w"]],
                            d2[:, 0:r["w"]], 0.0, None, op0=ALU.max)
                        drained_hi = max(drained_hi, r["off"] + r["w"])

                    n_d_after = sum(1 for rr in rounds[ri + 1:] if rr["eng"] == "dve")
                    if cfg.get("defer_d") and n_d_after >= cfg.get("nodefer_last", 0):
                        pending_mm = pending_mm + mms
                        pending_drain.append(_drain)
                    else:
                        for (dd, loc, ft, a0, cw) in mms:
                            nc.tensor.matmul(
                                dd[:, loc:loc + cw],
                                pk[:, ft * 128:(ft + 1) * 128],
                                pk[:, NF_LOC + a0:NF_LOC + a0 + cw],
                                start=True, stop=True,
                            )
                        _drain()
                eff_lag = (0 if ri >= len(rounds) - 2
                           else 1 if ri >= len(rounds) - cfg["tail_lag_rounds"] else lag)
                if ri >= eff_lag:
                    cut = ri - eff_lag
                    ua = max([rr["off"] + rr["w"] for rr in rounds[:cut + 1]
                              if rr["eng"] == "act"], default=0)
                    ud = max([rr["off"] + rr["w"] for rr in rounds[:cut + 1]
                              if rr["eng"] == "dve"], default=0)
                    flush_4x(ua, min(ud, drained_hi))
            for (dd, loc, ft, a0, cw) in pending_mm:
                nc.tensor.matmul(
                    dd[:, loc:loc + cw],
                    pk[:, ft * 128:(ft + 1) * 128],
                    pk[:, NF_LOC + a0:NF_LOC + a0 + cw],
                    start=True, stop=True,
                )
            for fn in pending_drain:
                fn()
            flush_4x(XA, XD)

            # column sums via 1-cycle ones-matmuls, fenced behind the grid,
            # emitted in round order so late rounds' groups come last
            acc = psum.tile([128, NJ], f32, tag="d2")
            tc.no_sync_barrier()
            order = []
            for r in rounds:
                base = r["off"] if r["eng"] == "act" else XA + r["off"]
                for g in range(r["w"] // 128):
                    order.append(base // 128 + g)
            assert sorted(order) == list(range(NJ))
            for j in order:
                nc.tensor.matmul(
                    acc[:, j:j + 1],
                    c[:, j * 128:(j + 1) * 128],
                    ones[:, 0:1],
                    start=True, stop=True,
                )

            # split copy+DMA: bulk early, last groups in a small late pair
            if cfg["copy_split"] <= 0:
                j0 = 0
            else:
                ncut = max(n for n in (NJ - cfg["copy_split"],) if n > 0)
                j0 = min(order[ncut:])
            if j0 > 0:
                nc.scalar.copy(accs[:, 0:j0], acc[:, 0:j0])
                nc.sync.dma_start(out=out_d[:, 0:j0], in_=accs[:, 0:j0])
            if cfg.get("copy2_dve"):
                nc.vector.tensor_scalar(accs[:, j0:NJ], acc[:, j0:NJ],
                                        0.0, None, op0=ALU.bypass)
            else:
                nc.scalar.copy(accs[:, j0:NJ], acc[:, j0:NJ])
            nc.sync.dma_start(out=out_d[:, j0:128], in_=accs[:, j0:128])

    nc.compile()
    nc.finalize()
    return nc


def group_kinds(cfg=None):
    """Which 128-col ones-matmul groups are bit-hack groups (True)."""
    cfg = cfg or DEFAULT_CFG
    _, XA, XD = _layout(cfg)
    return np.array([j * 128 >= XA for j in range(STREAM // 128)], bool)


def _features(predicted_frames_R, predicted_frames_t, predicted_atom_positions,
              atom_mask, true_frames_R, true_frames_t, true_atom_positions,
              seq_mask):
    """Host-side O(N+F) feature build. Returns A (NA,K), B (NF,K), counts."""
    f32 = np.float32
    Rp = np.asarray(predicted_frames_R, f32).reshape(NF, 3, 3)
    tp = np.asarray(predicted_frames_t, f32).reshape(NF, 3)
    Rt = np.asarray(true_frames_R, f32).reshape(NF, 3, 3)
    tt = np.asarray(true_frames_t, f32).reshape(NF, 3)
    p = np.asarray(predicted_atom_positions, f32).reshape(NA, 3)
    q = np.asarray(true_atom_positions, f32).reshape(NA, 3)
    m = (np.asarray(atom_mask, f32) * np.asarray(seq_mask, f32)[:, :, None]).reshape(NA)
    fm = np.broadcast_to(
        np.asarray(seq_mask, f32)[:, :, None], (B_, N_, F_)).reshape(NF)

    pp = np.einsum('aj,ak->ajk', p, p).reshape(NA, 9)
    qq = np.einsum('aj,ak->ajk', q, q).reshape(NA, 9)
    pq = np.einsum('aj,ak->ajk', p, q).reshape(NA, 9)
    Afeat = np.concatenate(
        [pp, qq, pq, p, q, np.ones((NA, 1), f32)], axis=1) * (m ** 2)[:, None]

    Gp = np.einsum('fij,fkj->fik', Rp, Rp)
    Gt = np.einsum('fij,fkj->fik', Rt, Rt)
    M = np.einsum('fij,fkj->fik', Rp, Rt)
    vec_p = -2 * np.einsum('fjk,fk->fj', Gp, tp) + 2 * np.einsum('fjk,fk->fj', M, tt)
    vec_q = -2 * np.einsum('fjk,fk->fj', Gt, tt) + 2 * np.einsum('fkj,fk->fj', M, tp)
    const = (np.einsum('fj,fjk,fk->f', tp, Gp, tp)
             + np.einsum('fj,fjk,fk->f', tt, Gt, tt)
             - 2 * np.einsum('fj,fjk,fk->f', tp, M, tt) + EPS)
    Bfeat = np.concatenate(
        [Gp.reshape(NF, 9), Gt.reshape(NF, 9), -2 * M.reshape(NF, 9),
         vec_p, vec_q, const[:, None]], axis=1)
    Bfeat = Bfeat * fm[:, None] * np.float32(K2)

    ac = max(float(m.sum()), 1.0)
    fc = max(float(fm.sum()), 1.0)
    return Afeat, Bfeat, ac, fc


def make_in_maps(inputs):
    Afeat, Bfeat, ac, fc = _features(**inputs)
    f32 = np.float32
    aT = np.ascontiguousarray(Afeat.T.astype(f32))            # (K, NA)
    in_maps = []
    for cix in range(NCORES):
        Bc = Bfeat[cix * NF_LOC:(cix + 1) * NF_LOC]           # (NF_LOC, K)
        pk = np.concatenate([Bc.T.astype(f32), aT], axis=1)   # (K, NF_LOC+NA)
        in_maps.append({"pk": np.ascontiguousarray(pk)})
    return in_maps, ac, fc


def _build_fast_exec(nc):
    """Cache the jitted 8-core executable so repeat kernel() calls skip jax
    re-tracing. Mirrors bass2jax.run_bass_via_pjrt's multi-core path."""
    import jax
    from concourse import bass2jax
    from jax.experimental.shard_map import shard_map
    from jax.sharding import Mesh, PartitionSpec

    bass2jax.install_neuronx_cc_hook()
    partition_name = nc.partition_id_tensor.name if nc.partition_id_tensor else None

    in_names, out_names, out_avals, zero_shapes = [], [], [], []
    for alloc in nc.m.functions[0].allocations:
        if not isinstance(alloc, mybir.MemoryLocationSet):
            continue
        name = alloc.memorylocations[0].name
        if alloc.kind == "ExternalInput":
            if name != partition_name:
                in_names.append(name)
        elif alloc.kind == "ExternalOutput":
            shape = tuple(alloc.tensor_shape)
            dtype = mybir.dt.np(alloc.dtype)
            out_names.append(name)
            out_avals.append(jax.core.ShapedArray(shape, dtype))
            zero_shapes.append((shape, dtype))
    n_params = len(in_names)
    all_names = in_names + out_names + ([partition_name] if partition_name else [])
    donate = tuple(range(n_params, n_params + len(out_names)))

    def _body(*args):
        operands = list(args)
        if partition_name is not None:
            operands.append(bass2jax.partition_id_tensor())
        return tuple(bass2jax._bass_exec_p.bind(
            *operands,
            out_avals=tuple(out_avals),
            in_names=tuple(all_names),
            out_names=tuple(out_names),
            lowering_input_output_aliases=(),
            sim_require_finite=True,
            sim_require_nnan=True,
            nc=nc,
        ))

    devices = jax.devices()[:NCORES]
    mesh = Mesh(np.asarray(devices), ("core",))
    specs = (PartitionSpec("core"),) * (n_params + len(out_names))
    sharded = jax.jit(
        shard_map(_body, mesh=mesh, in_specs=specs,
                  out_specs=(PartitionSpec("core"),) * len(out_names),
                  check_rep=False),
        donate_argnums=donate, keep_unused=True,
    )

    def run(in_maps):
        concat_in = [
            np.concatenate([np.asarray(m[k]) for m in in_maps], axis=0)
            for k in in_names
        ]
        concat_zeros = [
            np.zeros((NCORES * s[0], *s[1:]), dt) for (s, dt) in zero_shapes
        ]
        outs = sharded(*concat_in, *concat_zeros)
        return [
            {name: np.asarray(outs[i]).reshape(NCORES, *zero_shapes[i][0])[c]
             for i, name in enumerate(out_names)}
            for c in range(NCORES)
        ]

    return run


def kernel(**inputs) -> np.ndarray:
    in_maps, ac, fc = make_in_maps(inputs)

    if "nc" not in _cache:
        _cache["nc"] = _build_graph()
    nc = _cache["nc"]

    results = None
    try:
        if "fast" not in _cache:
            _cache["fast"] = _build_fast_exec(nc)
        results = _cache["fast"](in_maps)
    except Exception:
        _cache.pop("fast", None)
        results = run_bass_kernel_spmd(
            nc, in_maps, core_ids=list(range(NCORES))).results

    total = reduce_outputs(results)
    loss = total / (ac * fc * Z)
    return np.array([loss], np.float32)


def reduce_outputs(results) -> float:
    """Sum per-core group partials; bit-hack groups scaled by GAMMA."""
    kinds = group_kinds()
    NJ = len(kinds)
    total = 0.0
    for r in results:
        g = np.asarray(r["out"], np.float64)[:, :NJ].sum(axis=0)
        total += g[~kinds].sum() + GAMMA * g[kinds].sum()
    return total
